# revision 10
# baseline (speedup 1.0000x reference)
"""Causal self-attention (B=4, T=2048, C=1024, H=16) on 8 trn2 NeuronCores.

Sharding: tensor-parallel over heads. Core c owns heads {2c, 2c+1}:
  - computes Q,K,V projections for its 2 heads (full batch/sequence),
  - causal attention for its heads,
  - a partial output projection (row-slice of W_proj),
and the host sums the 8 partial projections (+ b_proj).

v4 design (v1 baseline 731us, v3 551us):
  - q-windows of 512 with ki-PAIRED S^T PSUM tiles: stt [128,1024] holds two
    k-tiles' S (512 q columns each) so one ScalarE exp instruction covers a
    k-tile pair (halves ACT instruction overhead vs 512-wide exps) while a
    single stt costs 2 PSUM banks.  PSUM budget: st 2x2 + av 2 + qkv 2 = 8.
  - st bufs=2 gives 4 k-tiles of S/exp lookahead; AV matmuls (lhsT =
    token-major V_aug bf16 [tok,65]; ones column = softmax denominator) lag
    one pair behind so the PE never queues behind ACT.
  - QKV projection is interleaved INTO the attention loop: window i emits
    the QKV chunks for token-tile i+1 (matmul work that fills every PE
    dependency stall and keeps HAM at K=8/8 -- v1/v3 ran attention at
    1.2GHz because PE idle gaps kept it throttled).
  - S matmuls alternate heads (row groups 0-1/2-3 via base partitions) for
    PE row-tiling overlap; exp output is bf16; causal masking is a 0/1
    multiply on the diagonal 128-block after exp, alternating DVE/gpsimd;
    out-projection PSUM->SBUF copies alternate DVE/ScalarE.
  - normalize: l row -> base-0 tile (cross-base 1-partition DVE copy),
    reciprocal_approx_fast, gpsimd partition_broadcast, one DVE mul per
    head (DVE handles the head-1 base-64 output directly).
"""

import numpy as np

import concourse.bacc as bacc
import concourse.tile as tile
from concourse import mybir
from concourse.bass_utils import run_bass_kernel_spmd
from concourse.masks import make_identity

F32 = mybir.dt.float32
F32R = mybir.dt.float32r
BF16 = mybir.dt.bfloat16
AF = mybir.ActivationFunctionType
ALU = mybir.AluOpType

N_CORES = 8
D_MODEL = 1024
N_HEADS = 16
HEAD_DIM = 64
H_LOC = 2                  # heads per core
D_LOC = H_LOC * HEAD_DIM   # 128
SCALE = 1.0 / np.sqrt(HEAD_DIM)
WIN = 512                  # q-window width


def build_program(B=4, T=2048):
    TOK = B * T
    TT = TOK // 512          # tok tiles of 512 for the QKV matmul
    CT = D_MODEL // 128      # contraction tiles
    NW = T // WIN            # q-windows per batch
    assert T % WIN == 0 and TOK % 512 == 0

    nc = bacc.Bacc(
        "TRN2", target_bir_lowering=False, debug=False, num_devices=N_CORES
    )
    xT = nc.dram_tensor("xT", [D_MODEL, TOK], F32R, kind="ExternalInput").ap()
    wq = nc.dram_tensor("wq", [D_MODEL, D_LOC], F32R, kind="ExternalInput").ap()
    wk = nc.dram_tensor("wk", [D_MODEL, D_LOC], F32R, kind="ExternalInput").ap()
    wv = nc.dram_tensor("wv", [D_MODEL, D_LOC], F32R, kind="ExternalInput").ap()
    bq = nc.dram_tensor("bq", [D_LOC, 1], F32, kind="ExternalInput").ap()
    bk = nc.dram_tensor("bk", [D_LOC, 1], F32, kind="ExternalInput").ap()
    bv = nc.dram_tensor("bv", [D_LOC, 1], F32, kind="ExternalInput").ap()
    wp = nc.dram_tensor("wp", [D_LOC, D_MODEL], F32R, kind="ExternalInput").ap()
    outp = nc.dram_tensor("outp", [TOK, D_MODEL], F32, kind="ExternalOutput").ap()

    with tile.TileContext(nc) as tc:
        with (
            tc.tile_pool(name="const", bufs=1) as const,
            tc.tile_pool(name="res", bufs=1) as res,
            tc.tile_pool(name="xst", bufs=10) as xst,
            tc.tile_pool(name="vtt", bufs=2) as vtt,
            tc.tile_pool(name="ptp", bufs=6) as ptpool,
            tc.tile_pool(name="m2", bufs=2) as m2,
            tc.tile_pool(name="otw", bufs=2) as otwp,
            tc.tile_pool(name="ob", bufs=4) as obp,
            tc.tile_pool(name="ps", bufs=1, space="PSUM") as ps,
        ):
            # --- constants -------------------------------------------------
            wq_sb = const.tile([128, CT, D_LOC], F32R, tag="wq")
            wk_sb = const.tile([128, CT, D_LOC], F32R, tag="wk")
            wv_sb = const.tile([128, CT, D_LOC], F32R, tag="wv")
            for w_sb, w_dram in ((wq_sb, wq), (wk_sb, wk), (wv_sb, wv)):
                nc.sync.dma_start(
                    out=w_sb, in_=w_dram.rearrange("(ct p) d -> p ct d", p=128)
                )
            wp_sb = const.tile([128, D_MODEL], F32R, tag="wp")
            nc.sync.dma_start(out=wp_sb, in_=wp)
            bq_sb = const.tile([128, 1], F32, tag="bq")
            bk_sb = const.tile([128, 1], F32, tag="bk")
            bv_sb = const.tile([128, 1], F32, tag="bv")
            for b_sb, b_dram in ((bq_sb, bq), (bk_sb, bk), (bv_sb, bv)):
                nc.sync.dma_start(out=b_sb, in_=b_dram)

            # 0/1 causal mask for a diagonal S^T block: [k, q] -> 1 iff q>=k
            mask_f32 = const.tile([128, 128], F32, tag="mask_f32")
            nc.gpsimd.memset(mask_f32, 1.0)
            nc.gpsimd.affine_select(
                out=mask_f32, in_=mask_f32, compare_op=ALU.is_ge,
                fill=0.0, base=0, pattern=[[1, 128]], channel_multiplier=-1,
            )
            mask01 = const.tile([128, 128], BF16, tag="mask01")
            nc.vector.tensor_copy(mask01, mask_f32)
            ident_f32 = const.tile([128, 128], F32, tag="ident_f32")
            make_identity(nc, ident_f32)
            ident = const.tile([128, 128], F32R, tag="ident")
            nc.vector.tensor_copy(ident, ident_f32)
            ones_f32 = const.tile([128, 128], F32, tag="ones_f32")
            nc.vector.memset(ones_f32, 1.0)

            # --- resident tensors -----------------------------------------
            qt_s = res.tile([128, TOK], F32R, tag="qt")   # [d(2 heads), tok]
            kt_s = res.tile([128, TOK], F32R, tag="kt")
            # token-major V with ones column: [tok(128), head, blk, 65] bf16
            vtm = res.tile([128, H_LOC, TOK // 128, 65], BF16, tag="vtm")
            nc.vector.tensor_copy(
                vtm[:, :, :, 64],
                ones_f32.rearrange("p (h b) -> p h b", h=H_LOC)[:, :, :TOK // 128],
            )

            # ---------------- QKV chunk machinery -------------------------
            # Each token-tile tt = 512 tokens; 4 chunks: q / k / v+vt / 4
            # transposes+vtm copy.  PSUM tag "qk" (2 banks) is exclusive to
            # this machinery, so it never deadlocks against st/av rings.
            xs_tiles = {}

            def emit_x_dma(tt):
                t0 = tt * 512
                xs = []
                for ct in range(CT):
                    xt = xst.tile([128, 512], F32R, tag="x",
                                  name=f"x_{tt}_{ct}")
                    nc.sync.dma_start(
                        out=xt, in_=xT[ct * 128:(ct + 1) * 128, t0:t0 + 512]
                    )
                    xs.append(xt)
                xs_tiles[tt] = xs

            xs_vt = {}

            def qkv_chunk(tt, c):
                t0 = tt * 512
                xs = xs_tiles[tt]
                if c < 3:
                    w_sb = (wq_sb, wk_sb, wv_sb)[c]
                    acc = ps.tile([128, 512], F32, tag="qk", bufs=2,
                                  name=f"p{'qkv'[c]}_{tt}")
                    for ct in range(CT):
                        nc.tensor.matmul(acc, w_sb[:, ct, :], xs[ct],
                                         start=ct == 0, stop=ct == CT - 1)
                    if c == 0:
                        nc.vector.tensor_scalar_add(
                            qt_s[:, t0:t0 + 512], acc, bq_sb)
                    elif c == 1:
                        nc.vector.tensor_scalar_add(
                            kt_s[:, t0:t0 + 512], acc, bk_sb)
                    else:
                        vt = vtt.tile([128, 512], F32R, tag="vt",
                                      name=f"vt_{tt}")
                        nc.vector.tensor_scalar_add(vt, acc, bv_sb)
                        xs_vt[tt] = vt
                else:
                    vt = xs_vt.pop(tt)
                    tp4 = ps.tile([128, 512], F32R, tag="qk", bufs=2,
                                  name=f"tp_{tt}")
                    for j in range(4):
                        nc.tensor.transpose(
                            tp4[:, j * 128:(j + 1) * 128],
                            vt[:, j * 128:(j + 1) * 128], ident,
                        )
                    blk0 = tt * 4
                    nc.vector.tensor_copy(
                        vtm[:, :, blk0:blk0 + 4, 0:64],
                        tp4.rearrange("p (j h d) -> p h j d", j=4, h=H_LOC),
                    )
                    del xs_tiles[tt]

            # ---------------- attention window machinery ------------------
            nmask = [0]
            nob = [0]

            def emit_s_exp_pair(b, w, k0, h, sttd):
                """S matmuls for k-tiles (k0, k0+1) of head h + one exp."""
                g0 = b * T + w * WIN
                hd = h * 64
                stt = ps.tile([128, 1024], F32, tag="st", bufs=2,
                              name=f"st_{b}_{w}_{k0}_{h}")
                offs = []
                for idx in range(2):
                    ki = k0 + idx
                    off = max(0, ki * 128 - w * WIN)
                    kg = b * T + ki * 128
                    nc.tensor.matmul(
                        stt[:, idx * 512 + off:(idx + 1) * 512],
                        kt_s[hd:hd + 64, kg:kg + 128],
                        qt_s[hd:hd + 64, g0 + off:g0 + 512],
                        start=True, stop=True,
                    )
                    offs.append(off)
                ptt = ptpool.tile([128, 1024], BF16, tag="pt",
                                  name=f"pt_{b}_{w}_{k0}_{h}")
                nc.scalar.activation(
                    ptt[:, offs[0]:1024], stt[:, offs[0]:1024],
                    AF.Exp, scale=SCALE,
                )
                for idx in range(2):
                    off = offs[idx]
                    if k0 + idx >= 4 * w:   # diagonal 128-block -> mask
                        eng = nc.vector if nmask[0] % 2 == 0 else nc.gpsimd
                        nmask[0] += 1
                        lo = idx * 512 + off
                        eng.tensor_mul(
                            ptt[:, lo:lo + 128], ptt[:, lo:lo + 128], mask01
                        )
                sttd[k0, h] = (offs, ptt)

            def emit_av_pair(b, w, k0, h, avs, sttd, nk):
                offs, ptt = sttd.pop((k0, h))
                for idx in range(2):
                    ki = k0 + idx
                    off = offs[idx]
                    blk = (b * T) // 128 + ki
                    nc.tensor.matmul(
                        avs[h][0:65, off:512],
                        vtm[:, h, blk, :],
                        ptt[:, idx * 512 + off:(idx + 1) * 512],
                        start=(ki == 0), stop=(ki == nk - 1),
                    )

            def emit_tail(b, w, avs):
                """normalize by l, out-projection, store one 512-window."""
                g0 = b * T + w * WIN
                ot_w = otwp.tile([128, 512], F32R, tag="ot",
                                 name=f"ot_{b}_{w}")
                for h in range(H_LOC):
                    ap = avs[h]
                    lrow = m2.tile([1, 512], F32, tag="lrow",
                                   name=f"lr_{b}_{w}_{h}")
                    nc.vector.tensor_copy(lrow, ap[64:65, :])
                    lrinv = m2.tile([1, 512], F32, tag="lrinv",
                                    name=f"lv_{b}_{w}_{h}")
                    nc.vector.reciprocal_approx_fast(out=lrinv, in_=lrow)
                    linv = m2.tile([64, 512], F32, tag="linv",
                                   name=f"li_{b}_{w}_{h}")
                    nc.gpsimd.partition_broadcast(linv, lrinv)
                    nc.vector.tensor_mul(
                        ot_w[h * 64:h * 64 + 64, :], ap[0:64, :], linv
                    )
                for ti in range(4):
                    t0 = g0 + ti * 128
                    for co in range(2):
                        po = ps.tile([128, 512], F32, tag="qk", bufs=2,
                                     name=f"po_{b}_{w}_{ti}_{co}")
                        nc.tensor.matmul(
                            po, ot_w[:, ti * 128:ti * 128 + 128],
                            wp_sb[:, co * 512:(co + 1) * 512],
                            start=True, stop=True,
                        )
                        ob = obp.tile([128, 512], F32, tag="ob")
                        if nob[0] % 2 == 0:
                            nc.vector.tensor_copy(ob, po)
                        else:
                            nc.scalar.activation(ob, po, AF.Copy)
                        nob[0] += 1
                        nc.sync.dma_start(
                            out=outp[t0:t0 + 128, co * 512:(co + 1) * 512],
                            in_=ob,
                        )

            # ---------------- main interleaved schedule -------------------
            emit_x_dma(0)
            for c in range(4):
                qkv_chunk(0, c)

            windows = [(b, w) for b in range(B) for w in range(NW)]
            prev_tail = None
            for i, (b, w) in enumerate(windows):
                nk = 4 * (w + 1)
                npairs = nk // 2
                if i + 1 < TT:
                    emit_x_dma(i + 1)
                chunks = [(i + 1, c) for c in range(4)] if i + 1 < TT else []
                avs = {}
                sttd = {}
                pend = []
                for j in range(npairs):
                    k0 = 2 * j
                    for h in range(H_LOC):
                        emit_s_exp_pair(b, w, k0, h, sttd)
                    if chunks:
                        qkv_chunk(*chunks.pop(0))
                    if j == min(2, npairs - 1) and prev_tail is not None:
                        emit_tail(*prev_tail)
                        prev_tail = None
                    if j >= 1:
                        if not avs:
                            for h in range(H_LOC):
                                avs[h] = ps.tile(
                                    [128, 512], F32, tag="av", bufs=2,
                                    name=f"av_{b}_{w}_{h}",
                                )
                        pk0 = 2 * (j - 1)
                        for h in range(H_LOC):
                            emit_av_pair(b, w, pk0, h, avs, sttd, nk)
                if prev_tail is not None:   # npairs == 2 path never hits j>1
                    emit_tail(*prev_tail)
                    prev_tail = None
                if not avs:
                    for h in range(H_LOC):
                        avs[h] = ps.tile([128, 512], F32, tag="av", bufs=2,
                                         name=f"av_{b}_{w}_{h}")
                for h in range(H_LOC):
                    emit_av_pair(b, w, 2 * (npairs - 1), h, avs, sttd, nk)
                while chunks:
                    qkv_chunk(*chunks.pop(0))
                prev_tail = (b, w, avs)
            emit_tail(*prev_tail)
    nc.compile()
    return nc


_PROGRAM = None


def _get_program():
    global _PROGRAM
    if _PROGRAM is None:
        _PROGRAM = build_program()
    return _PROGRAM


def _make_in_maps(x, W_qkv, b_qkv, W_proj):
    B, T, C = x.shape
    xT = np.ascontiguousarray(
        x.reshape(B * T, C).T.astype(np.float32)
    )
    in_maps = []
    for c in range(N_CORES):
        lo, hi = c * D_LOC, (c + 1) * D_LOC
        in_maps.append({
            "xT": xT,
            "wq": np.ascontiguousarray(W_qkv[:, lo:hi], np.float32),
            "wk": np.ascontiguousarray(W_qkv[:, C + lo:C + hi], np.float32),
            "wv": np.ascontiguousarray(W_qkv[:, 2 * C + lo:2 * C + hi], np.float32),
            "bq": np.ascontiguousarray(b_qkv[lo:hi].reshape(-1, 1), np.float32),
            "bk": np.ascontiguousarray(b_qkv[C + lo:C + hi].reshape(-1, 1), np.float32),
            "bv": np.ascontiguousarray(b_qkv[2 * C + lo:2 * C + hi].reshape(-1, 1), np.float32),
            "wp": np.ascontiguousarray(W_proj[lo:hi, :], np.float32),
        })
    return in_maps


LAST_RESULT = None


def run(inputs, trace=False):
    """Returns (full output [B,T,C] float32, exec_time_ns or None)."""
    global LAST_RESULT
    x = np.asarray(inputs["x"], np.float32)
    W_qkv = np.asarray(inputs["W_qkv"], np.float32)
    b_qkv = np.asarray(inputs["b_qkv"], np.float32)
    W_proj = np.asarray(inputs["W_proj"], np.float32)
    b_proj = np.asarray(inputs["b_proj"], np.float32)
    B, T, C = x.shape

    nc = _get_program()
    in_maps = _make_in_maps(x, W_qkv, b_qkv, W_proj)
    res = run_bass_kernel_spmd(
        nc, in_maps, list(range(N_CORES)), trace=trace
    )
    LAST_RESULT = res
    acc = np.zeros((B * T, C), np.float64)
    for c in range(N_CORES):
        acc += res.results[c]["outp"].astype(np.float64)
    out = (acc + b_proj.astype(np.float64)).astype(np.float32)
    return out.reshape(B, T, C), res.exec_time_ns


def kernel(**inputs):
    out, _ = run(inputs, trace=False)
    return out


# revision 11
# speedup vs baseline: 1.1374x; 1.1374x over previous
"""Causal self-attention (B=4, T=2048, C=1024, H=16) on 8 trn2 NeuronCores.

Sharding: tensor-parallel over heads. Core c owns heads {2c, 2c+1}:
  - computes Q,K,V projections for its 2 heads (full batch/sequence),
  - causal attention for its heads,
  - a partial output projection (row-slice of W_proj),
and the host sums the 8 partial projections (+ b_proj).

v4 design (v1 baseline 731us, v3 551us):
  - q-windows of 512 with ki-PAIRED S^T PSUM tiles: stt [128,1024] holds two
    k-tiles' S (512 q columns each) so one ScalarE exp instruction covers a
    k-tile pair (halves ACT instruction overhead vs 512-wide exps) while a
    single stt costs 2 PSUM banks.  PSUM budget: st 2x2 + av 2 + qkv 2 = 8.
  - st bufs=2 gives 4 k-tiles of S/exp lookahead; AV matmuls (lhsT =
    token-major V_aug bf16 [tok,65]; ones column = softmax denominator) lag
    one pair behind so the PE never queues behind ACT.
  - QKV projection is interleaved INTO the attention loop: window i emits
    the QKV chunks for token-tile i+1 (matmul work that fills every PE
    dependency stall and keeps HAM at K=8/8 -- v1/v3 ran attention at
    1.2GHz because PE idle gaps kept it throttled).
  - S matmuls alternate heads (row groups 0-1/2-3 via base partitions) for
    PE row-tiling overlap; exp output is bf16; causal masking is a 0/1
    multiply on the diagonal 128-block after exp, alternating DVE/gpsimd;
    out-projection PSUM->SBUF copies alternate DVE/ScalarE.
  - normalize: l row -> base-0 tile (cross-base 1-partition DVE copy),
    reciprocal_approx_fast, gpsimd partition_broadcast, one DVE mul per
    head (DVE handles the head-1 base-64 output directly).
"""

import numpy as np

import concourse.bacc as bacc
import concourse.tile as tile
from concourse import mybir
from concourse.bass_utils import run_bass_kernel_spmd
from concourse.masks import make_identity

F32 = mybir.dt.float32
F32R = mybir.dt.float32r
BF16 = mybir.dt.bfloat16
AF = mybir.ActivationFunctionType
ALU = mybir.AluOpType

N_CORES = 8
D_MODEL = 1024
N_HEADS = 16
HEAD_DIM = 64
H_LOC = 2                  # heads per core
D_LOC = H_LOC * HEAD_DIM   # 128
SCALE = 1.0 / np.sqrt(HEAD_DIM)
WIN = 512                  # q-window width


def build_program(B=4, T=2048):
    TOK = B * T
    TT = TOK // 512          # tok tiles of 512 for the QKV matmul
    CT = D_MODEL // 128      # contraction tiles
    NW = T // WIN            # q-windows per batch
    assert T % WIN == 0 and TOK % 512 == 0

    nc = bacc.Bacc(
        "TRN2", target_bir_lowering=False, debug=False, num_devices=N_CORES
    )
    xT = nc.dram_tensor("xT", [D_MODEL, TOK], F32R, kind="ExternalInput").ap()
    wq = nc.dram_tensor("wq", [D_MODEL, D_LOC], F32R, kind="ExternalInput").ap()
    wk = nc.dram_tensor("wk", [D_MODEL, D_LOC], F32R, kind="ExternalInput").ap()
    wv = nc.dram_tensor("wv", [D_MODEL, D_LOC], F32R, kind="ExternalInput").ap()
    bq = nc.dram_tensor("bq", [D_LOC, 1], F32, kind="ExternalInput").ap()
    bk = nc.dram_tensor("bk", [D_LOC, 1], F32, kind="ExternalInput").ap()
    bv = nc.dram_tensor("bv", [D_LOC, 1], F32, kind="ExternalInput").ap()
    wp = nc.dram_tensor("wp", [D_LOC, D_MODEL], F32R, kind="ExternalInput").ap()
    outp = nc.dram_tensor("outp", [TOK, D_MODEL], F32, kind="ExternalOutput").ap()

    with tile.TileContext(nc) as tc:
        with (
            tc.tile_pool(name="const", bufs=1) as const,
            tc.tile_pool(name="res", bufs=1) as res,
            tc.tile_pool(name="xst", bufs=10) as xst,
            tc.tile_pool(name="vtt", bufs=2) as vtt,
            tc.tile_pool(name="ptp", bufs=6) as ptpool,
            tc.tile_pool(name="m2", bufs=2) as m2,
            tc.tile_pool(name="otw", bufs=2) as otwp,
            tc.tile_pool(name="ob", bufs=4) as obp,
            tc.tile_pool(name="ps", bufs=1, space="PSUM") as ps,
        ):
            # --- constants -------------------------------------------------
            wq_sb = const.tile([128, CT, D_LOC], F32R, tag="wq")
            wk_sb = const.tile([128, CT, D_LOC], F32R, tag="wk")
            wv_sb = const.tile([128, CT, D_LOC], F32R, tag="wv")
            for w_sb, w_dram in ((wq_sb, wq), (wk_sb, wk), (wv_sb, wv)):
                nc.sync.dma_start(
                    out=w_sb, in_=w_dram.rearrange("(ct p) d -> p ct d", p=128)
                )
            wp_sb = const.tile([128, D_MODEL], F32R, tag="wp")
            nc.sync.dma_start(out=wp_sb, in_=wp)
            bq_sb = const.tile([128, 1], F32, tag="bq")
            bk_sb = const.tile([128, 1], F32, tag="bk")
            bv_sb = const.tile([128, 1], F32, tag="bv")
            for b_sb, b_dram in ((bq_sb, bq), (bk_sb, bk), (bv_sb, bv)):
                nc.sync.dma_start(out=b_sb, in_=b_dram)

            # causal mask for a diagonal S^T block [k, q]: 0 where q >= k,
            # -1e9 where q < k.  Applied ON THE PE as an accumulated matmul
            # (ident.T @ tri) so the hot loop never round-trips DVE/gpsimd.
            mask_f32 = const.tile([128, 128], F32, tag="mask_f32")
            nc.gpsimd.memset(mask_f32, 0.0)
            nc.gpsimd.affine_select(
                out=mask_f32, in_=mask_f32, compare_op=ALU.is_ge,
                fill=-1.0e9, base=0, pattern=[[1, 128]], channel_multiplier=-1,
            )
            tri_r = const.tile([128, 128], F32R, tag="tri_r")
            nc.vector.tensor_copy(tri_r, mask_f32)
            ident_f32 = const.tile([128, 128], F32, tag="ident_f32")
            make_identity(nc, ident_f32)
            ident = const.tile([128, 128], F32R, tag="ident")
            nc.vector.tensor_copy(ident, ident_f32)
            ones_f32 = const.tile([128, 128], F32, tag="ones_f32")
            nc.vector.memset(ones_f32, 1.0)

            # --- resident tensors -----------------------------------------
            qt_s = res.tile([128, TOK], F32R, tag="qt")   # [d(2 heads), tok]
            kt_s = res.tile([128, TOK], F32R, tag="kt")
            # token-major V with ones column: [tok(128), head, blk, 65] bf16
            vtm = res.tile([128, H_LOC, TOK // 128, 65], BF16, tag="vtm")
            nc.vector.tensor_copy(
                vtm[:, :, :, 64],
                ones_f32.rearrange("p (h b) -> p h b", h=H_LOC)[:, :, :TOK // 128],
            )

            # ---------------- QKV chunk machinery -------------------------
            # Each token-tile tt = 512 tokens; 4 chunks: q / k / v+vt / 4
            # transposes+vtm copy.  PSUM tag "qk" (2 banks) is exclusive to
            # this machinery, so it never deadlocks against st/av rings.
            xs_tiles = {}

            def emit_x_dma(tt):
                t0 = tt * 512
                xs = []
                for ct in range(CT):
                    xt = xst.tile([128, 512], F32R, tag="x",
                                  name=f"x_{tt}_{ct}")
                    nc.sync.dma_start(
                        out=xt, in_=xT[ct * 128:(ct + 1) * 128, t0:t0 + 512]
                    )
                    xs.append(xt)
                xs_tiles[tt] = xs

            xs_vt = {}

            def qkv_chunk(tt, c):
                t0 = tt * 512
                xs = xs_tiles[tt]
                if c < 3:
                    w_sb = (wq_sb, wk_sb, wv_sb)[c]
                    acc = ps.tile([128, 512], F32, tag="qk", bufs=2,
                                  name=f"p{'qkv'[c]}_{tt}")
                    for ct in range(CT):
                        nc.tensor.matmul(acc, w_sb[:, ct, :], xs[ct],
                                         start=ct == 0, stop=ct == CT - 1)
                    if c == 0:
                        nc.vector.tensor_scalar_add(
                            qt_s[:, t0:t0 + 512], acc, bq_sb)
                    elif c == 1:
                        nc.vector.tensor_scalar_add(
                            kt_s[:, t0:t0 + 512], acc, bk_sb)
                    else:
                        vt = vtt.tile([128, 512], F32R, tag="vt",
                                      name=f"vt_{tt}")
                        nc.vector.tensor_scalar_add(vt, acc, bv_sb)
                        xs_vt[tt] = vt
                else:
                    vt = xs_vt.pop(tt)
                    tp4 = ps.tile([128, 512], F32R, tag="qk", bufs=2,
                                  name=f"tp_{tt}")
                    for j in range(4):
                        nc.tensor.transpose(
                            tp4[:, j * 128:(j + 1) * 128],
                            vt[:, j * 128:(j + 1) * 128], ident,
                        )
                    blk0 = tt * 4
                    nc.vector.tensor_copy(
                        vtm[:, :, blk0:blk0 + 4, 0:64],
                        tp4.rearrange("p (j h d) -> p h j d", j=4, h=H_LOC),
                    )
                    del xs_tiles[tt]

            # ---------------- attention window machinery ------------------
            def emit_s_exp_pair(b, w, k0, h, sttd):
                """S matmuls for k-tiles (k0, k0+1) of head h + one exp."""
                g0 = b * T + w * WIN
                hd = h * 64
                stt = ps.tile([128, 1024], F32, tag="st", bufs=2,
                              name=f"st_{b}_{w}_{k0}_{h}")
                offs = []
                for idx in range(2):
                    ki = k0 + idx
                    off = max(0, ki * 128 - w * WIN)
                    kg = b * T + ki * 128
                    diag = ki >= 4 * w
                    nc.tensor.matmul(
                        stt[:, idx * 512 + off:(idx + 1) * 512],
                        kt_s[hd:hd + 64, kg:kg + 128],
                        qt_s[hd:hd + 64, g0 + off:g0 + 512],
                        start=True, stop=not diag,
                    )
                    if diag:  # add -1e9 upper triangle on the PE
                        nc.tensor.matmul(
                            stt[:, idx * 512 + off:idx * 512 + off + 128],
                            ident, tri_r, start=False, stop=True,
                        )
                    offs.append(off)
                ptt = ptpool.tile([128, 1024], BF16, tag="pt",
                                  name=f"pt_{b}_{w}_{k0}_{h}")
                nc.scalar.activation(
                    ptt[:, offs[0]:1024], stt[:, offs[0]:1024],
                    AF.Exp, scale=SCALE,
                )
                sttd[k0, h] = (offs, ptt)

            def emit_av_pair(b, w, k0, h, avs, sttd, nk):
                offs, ptt = sttd.pop((k0, h))
                for idx in range(2):
                    ki = k0 + idx
                    off = offs[idx]
                    blk = (b * T) // 128 + ki
                    nc.tensor.matmul(
                        avs[h][0:65, off:512],
                        vtm[:, h, blk, :],
                        ptt[:, idx * 512 + off:(idx + 1) * 512],
                        start=(ki == 0), stop=(ki == nk - 1),
                    )

            def emit_tail(b, w, avs):
                """normalize by l, out-projection, store one 512-window."""
                g0 = b * T + w * WIN
                ot_w = otwp.tile([128, 512], F32R, tag="ot",
                                 name=f"ot_{b}_{w}")
                for h in range(H_LOC):
                    ap = avs[h]
                    lrow = m2.tile([1, 512], F32, tag="lrow",
                                   name=f"lr_{b}_{w}_{h}")
                    nc.vector.tensor_copy(lrow, ap[64:65, :])
                    lrinv = m2.tile([1, 512], F32, tag="lrinv",
                                    name=f"lv_{b}_{w}_{h}")
                    nc.vector.reciprocal_approx_fast(out=lrinv, in_=lrow)
                    linv = m2.tile([64, 512], F32, tag="linv",
                                   name=f"li_{b}_{w}_{h}")
                    nc.gpsimd.partition_broadcast(linv, lrinv)
                    nc.vector.tensor_mul(
                        ot_w[h * 64:h * 64 + 64, :], ap[0:64, :], linv
                    )
                for ti in range(4):
                    t0 = g0 + ti * 128
                    for co in range(2):
                        po = ps.tile([128, 512], F32, tag="qk", bufs=2,
                                     name=f"po_{b}_{w}_{ti}_{co}")
                        nc.tensor.matmul(
                            po, ot_w[:, ti * 128:ti * 128 + 128],
                            wp_sb[:, co * 512:(co + 1) * 512],
                            start=True, stop=True,
                        )
                        ob = obp.tile([128, 512], F32, tag="ob")
                        nc.vector.tensor_copy(ob, po)
                        nc.sync.dma_start(
                            out=outp[t0:t0 + 128, co * 512:(co + 1) * 512],
                            in_=ob,
                        )

            # ---------------- main interleaved schedule -------------------
            emit_x_dma(0)
            for c in range(4):
                qkv_chunk(0, c)

            windows = [(b, w) for b in range(B) for w in range(NW)]
            prev_tail = None
            for i, (b, w) in enumerate(windows):
                nk = 4 * (w + 1)
                npairs = nk // 2
                if i + 1 < TT:
                    emit_x_dma(i + 1)
                chunks = [(i + 1, c) for c in range(4)] if i + 1 < TT else []
                avs = {}
                sttd = {}
                pend = []
                for j in range(npairs):
                    k0 = 2 * j
                    for h in range(H_LOC):
                        emit_s_exp_pair(b, w, k0, h, sttd)
                    if chunks:
                        qkv_chunk(*chunks.pop(0))
                    if j == min(2, npairs - 1) and prev_tail is not None:
                        emit_tail(*prev_tail)
                        prev_tail = None
                    if j >= 1:
                        if not avs:
                            for h in range(H_LOC):
                                avs[h] = ps.tile(
                                    [128, 512], F32, tag="av", bufs=2,
                                    name=f"av_{b}_{w}_{h}",
                                )
                        pk0 = 2 * (j - 1)
                        for h in range(H_LOC):
                            emit_av_pair(b, w, pk0, h, avs, sttd, nk)
                if prev_tail is not None:   # npairs == 2 path never hits j>1
                    emit_tail(*prev_tail)
                    prev_tail = None
                if not avs:
                    for h in range(H_LOC):
                        avs[h] = ps.tile([128, 512], F32, tag="av", bufs=2,
                                         name=f"av_{b}_{w}_{h}")
                for h in range(H_LOC):
                    emit_av_pair(b, w, 2 * (npairs - 1), h, avs, sttd, nk)
                while chunks:
                    qkv_chunk(*chunks.pop(0))
                prev_tail = (b, w, avs)
            emit_tail(*prev_tail)
    nc.compile()
    return nc


_PROGRAM = None


def _get_program():
    global _PROGRAM
    if _PROGRAM is None:
        _PROGRAM = build_program()
    return _PROGRAM


def _make_in_maps(x, W_qkv, b_qkv, W_proj):
    B, T, C = x.shape
    xT = np.ascontiguousarray(
        x.reshape(B * T, C).T.astype(np.float32)
    )
    in_maps = []
    for c in range(N_CORES):
        lo, hi = c * D_LOC, (c + 1) * D_LOC
        in_maps.append({
            "xT": xT,
            "wq": np.ascontiguousarray(W_qkv[:, lo:hi], np.float32),
            "wk": np.ascontiguousarray(W_qkv[:, C + lo:C + hi], np.float32),
            "wv": np.ascontiguousarray(W_qkv[:, 2 * C + lo:2 * C + hi], np.float32),
            "bq": np.ascontiguousarray(b_qkv[lo:hi].reshape(-1, 1), np.float32),
            "bk": np.ascontiguousarray(b_qkv[C + lo:C + hi].reshape(-1, 1), np.float32),
            "bv": np.ascontiguousarray(b_qkv[2 * C + lo:2 * C + hi].reshape(-1, 1), np.float32),
            "wp": np.ascontiguousarray(W_proj[lo:hi, :], np.float32),
        })
    return in_maps


LAST_RESULT = None


def run(inputs, trace=False):
    """Returns (full output [B,T,C] float32, exec_time_ns or None)."""
    global LAST_RESULT
    x = np.asarray(inputs["x"], np.float32)
    W_qkv = np.asarray(inputs["W_qkv"], np.float32)
    b_qkv = np.asarray(inputs["b_qkv"], np.float32)
    W_proj = np.asarray(inputs["W_proj"], np.float32)
    b_proj = np.asarray(inputs["b_proj"], np.float32)
    B, T, C = x.shape

    nc = _get_program()
    in_maps = _make_in_maps(x, W_qkv, b_qkv, W_proj)
    res = run_bass_kernel_spmd(
        nc, in_maps, list(range(N_CORES)), trace=trace
    )
    LAST_RESULT = res
    acc = np.zeros((B * T, C), np.float64)
    for c in range(N_CORES):
        acc += res.results[c]["outp"].astype(np.float64)
    out = (acc + b_proj.astype(np.float64)).astype(np.float32)
    return out.reshape(B, T, C), res.exec_time_ns


def kernel(**inputs):
    out, _ = run(inputs, trace=False)
    return out


# revision 13
# speedup vs baseline: 1.5894x; 1.3974x over previous
"""Causal self-attention (B=4, T=2048, C=1024, H=16) on 8 trn2 NeuronCores.

Sharding: tensor-parallel over heads. Core c owns heads {2c, 2c+1}:
  - computes Q,K,V projections for its 2 heads (full batch/sequence),
  - causal attention for its heads,
  - a partial output projection (row-slice of W_proj),
and the host sums the 8 partial projections (+ b_proj).

v4 design (v1 baseline 731us, v3 551us):
  - q-windows of 512 with ki-PAIRED S^T PSUM tiles: stt [128,1024] holds two
    k-tiles' S (512 q columns each) so one ScalarE exp instruction covers a
    k-tile pair (halves ACT instruction overhead vs 512-wide exps) while a
    single stt costs 2 PSUM banks.  PSUM budget: st 2x2 + av 2 + qkv 2 = 8.
  - st bufs=2 gives 4 k-tiles of S/exp lookahead; AV matmuls (lhsT =
    token-major V_aug bf16 [tok,65]; ones column = softmax denominator) lag
    one pair behind so the PE never queues behind ACT.
  - QKV projection is interleaved INTO the attention loop: window i emits
    the QKV chunks for token-tile i+1 (matmul work that fills every PE
    dependency stall and keeps HAM at K=8/8 -- v1/v3 ran attention at
    1.2GHz because PE idle gaps kept it throttled).
  - S matmuls alternate heads (row groups 0-1/2-3 via base partitions) for
    PE row-tiling overlap; exp output is bf16; causal masking is a 0/1
    multiply on the diagonal 128-block after exp, alternating DVE/gpsimd;
    out-projection PSUM->SBUF copies alternate DVE/ScalarE.
  - normalize: l row -> base-0 tile (cross-base 1-partition DVE copy),
    reciprocal_approx_fast, gpsimd partition_broadcast, one DVE mul per
    head (DVE handles the head-1 base-64 output directly).
"""

import ml_dtypes
import numpy as np

import concourse.bacc as bacc

BF16_NP = ml_dtypes.bfloat16
import concourse.tile as tile
from concourse import mybir
from concourse.bass_utils import run_bass_kernel_spmd
from concourse.masks import make_identity

F32 = mybir.dt.float32
F32R = mybir.dt.float32r
BF16 = mybir.dt.bfloat16
AF = mybir.ActivationFunctionType
ALU = mybir.AluOpType

N_CORES = 8
D_MODEL = 1024
N_HEADS = 16
HEAD_DIM = 64
H_LOC = 2                  # heads per core
D_LOC = H_LOC * HEAD_DIM   # 128
SCALE = 1.0 / np.sqrt(HEAD_DIM)
WIN = 512                  # q-window width


def build_program(B=4, T=2048):
    TOK = B * T
    TT = TOK // 512          # tok tiles of 512 for the QKV matmul
    CT = D_MODEL // 128      # contraction tiles
    NW = T // WIN            # q-windows per batch
    assert T % WIN == 0 and TOK % 512 == 0

    nc = bacc.Bacc(
        "TRN2", target_bir_lowering=False, debug=False, num_devices=N_CORES
    )
    xT = nc.dram_tensor("xT", [D_MODEL, TOK], BF16, kind="ExternalInput").ap()
    wq = nc.dram_tensor("wq", [D_MODEL, D_LOC], BF16, kind="ExternalInput").ap()
    wk = nc.dram_tensor("wk", [D_MODEL, D_LOC], BF16, kind="ExternalInput").ap()
    wv = nc.dram_tensor("wv", [D_MODEL, D_LOC], BF16, kind="ExternalInput").ap()
    bq = nc.dram_tensor("bq", [D_LOC, 1], F32, kind="ExternalInput").ap()
    bk = nc.dram_tensor("bk", [D_LOC, 1], F32, kind="ExternalInput").ap()
    bv = nc.dram_tensor("bv", [D_LOC, 1], F32, kind="ExternalInput").ap()
    wp = nc.dram_tensor("wp", [D_LOC, D_MODEL], BF16, kind="ExternalInput").ap()
    outp = nc.dram_tensor("outp", [TOK, D_MODEL], F32, kind="ExternalOutput").ap()

    with tile.TileContext(nc) as tc:
        with (
            tc.tile_pool(name="const", bufs=1) as const,
            tc.tile_pool(name="res", bufs=1) as res,
            tc.tile_pool(name="xst", bufs=10) as xst,
            tc.tile_pool(name="vtt", bufs=2) as vtt,
            tc.tile_pool(name="ptp", bufs=6) as ptpool,
            tc.tile_pool(name="m2", bufs=2) as m2,
            tc.tile_pool(name="otw", bufs=2) as otwp,
            tc.tile_pool(name="ob", bufs=4) as obp,
            tc.tile_pool(name="ps", bufs=1, space="PSUM") as ps,
        ):
            # --- constants -------------------------------------------------
            wq_sb = const.tile([128, CT, D_LOC], BF16, tag="wq")
            wk_sb = const.tile([128, CT, D_LOC], BF16, tag="wk")
            wv_sb = const.tile([128, CT, D_LOC], BF16, tag="wv")
            for w_sb, w_dram in ((wq_sb, wq), (wk_sb, wk), (wv_sb, wv)):
                nc.sync.dma_start(
                    out=w_sb, in_=w_dram.rearrange("(ct p) d -> p ct d", p=128)
                )
            wp_sb = const.tile([128, D_MODEL], BF16, tag="wp")
            nc.sync.dma_start(out=wp_sb, in_=wp)
            bq_sb = const.tile([128, 1], F32, tag="bq")
            bk_sb = const.tile([128, 1], F32, tag="bk")
            bv_sb = const.tile([128, 1], F32, tag="bv")
            for b_sb, b_dram in ((bq_sb, bq), (bk_sb, bk), (bv_sb, bv)):
                nc.sync.dma_start(out=b_sb, in_=b_dram)

            # causal mask for a diagonal S^T block [k, q]: 0 where q >= k,
            # -1e9 where q < k.  Applied ON THE PE as an accumulated matmul
            # (ident.T @ tri) so the hot loop never round-trips DVE/gpsimd.
            mask_f32 = const.tile([128, 128], F32, tag="mask_f32")
            nc.gpsimd.memset(mask_f32, 0.0)
            nc.gpsimd.affine_select(
                out=mask_f32, in_=mask_f32, compare_op=ALU.is_ge,
                fill=-1.0e9, base=0, pattern=[[1, 128]], channel_multiplier=-1,
            )
            tri_r = const.tile([128, 128], BF16, tag="tri_r")
            nc.vector.tensor_copy(tri_r, mask_f32)
            ident_f32 = const.tile([128, 128], F32, tag="ident_f32")
            make_identity(nc, ident_f32)
            ident = const.tile([128, 128], BF16, tag="ident")
            nc.vector.tensor_copy(ident, ident_f32)
            ident_r = const.tile([128, 128], F32R, tag="ident_r")
            nc.vector.tensor_copy(ident_r, ident_f32)
            ones_f32 = const.tile([128, 128], F32, tag="ones_f32")
            nc.vector.memset(ones_f32, 1.0)

            # --- resident tensors -----------------------------------------
            qt_s = res.tile([128, TOK], BF16, tag="qt")   # [d(2 heads), tok]
            kt_s = res.tile([128, TOK], BF16, tag="kt")
            # token-major V with ones column: [tok(128), head, blk, 65] bf16
            vtm = res.tile([128, H_LOC, TOK // 128, 65], BF16, tag="vtm")
            nc.vector.tensor_copy(
                vtm[:, :, :, 64],
                ones_f32.rearrange("p (h b) -> p h b", h=H_LOC)[:, :, :TOK // 128],
            )

            # ---------------- QKV chunk machinery -------------------------
            # Each token-tile tt = 512 tokens; 4 chunks: q / k / v+vt / 4
            # transposes+vtm copy.  PSUM tag "qk" (2 banks) is exclusive to
            # this machinery, so it never deadlocks against st/av rings.
            xs_tiles = {}

            def emit_x_dma(tt):
                t0 = tt * 512
                xs = []
                for ct in range(CT):
                    xt = xst.tile([128, 512], BF16, tag="x",
                                  name=f"x_{tt}_{ct}")
                    nc.sync.dma_start(
                        out=xt, in_=xT[ct * 128:(ct + 1) * 128, t0:t0 + 512]
                    )
                    xs.append(xt)
                xs_tiles[tt] = xs

            xs_vt = {}

            def qkv_chunk(tt, c):
                t0 = tt * 512
                xs = xs_tiles[tt]
                if c < 3:
                    w_sb = (wq_sb, wk_sb, wv_sb)[c]
                    acc = ps.tile([128, 512], F32, tag="qk", bufs=2,
                                  name=f"p{'qkv'[c]}_{tt}")
                    for ct in range(CT):
                        nc.tensor.matmul(acc, w_sb[:, ct, :], xs[ct],
                                         start=ct == 0, stop=ct == CT - 1)
                    if c == 0:
                        nc.vector.tensor_scalar_add(
                            qt_s[:, t0:t0 + 512], acc, bq_sb)
                    elif c == 1:
                        nc.vector.tensor_scalar_add(
                            kt_s[:, t0:t0 + 512], acc, bk_sb)
                    else:
                        vt = vtt.tile([128, 512], F32R, tag="vt",
                                      name=f"vt_{tt}")
                        nc.vector.tensor_scalar_add(vt, acc, bv_sb)
                        xs_vt[tt] = vt
                else:
                    vt = xs_vt.pop(tt)
                    tp4 = ps.tile([128, 512], F32R, tag="qk", bufs=2,
                                  name=f"tp_{tt}")
                    for j in range(4):
                        nc.tensor.transpose(
                            tp4[:, j * 128:(j + 1) * 128],
                            vt[:, j * 128:(j + 1) * 128], ident_r,
                        )
                    blk0 = tt * 4
                    nc.vector.tensor_copy(
                        vtm[:, :, blk0:blk0 + 4, 0:64],
                        tp4.rearrange("p (j h d) -> p h j d", j=4, h=H_LOC),
                    )
                    del xs_tiles[tt]

            # ---------------- attention window machinery ------------------
            def emit_s_exp_pair(b, w, k0, h, sttd):
                """S matmuls for k-tiles (k0, k0+1) of head h + one exp."""
                g0 = b * T + w * WIN
                hd = h * 64
                stt = ps.tile([128, 1024], F32, tag="st", bufs=2,
                              name=f"st_{b}_{w}_{k0}_{h}")
                offs = []
                for idx in range(2):
                    ki = k0 + idx
                    off = max(0, ki * 128 - w * WIN)
                    kg = b * T + ki * 128
                    diag = ki >= 4 * w
                    nc.tensor.matmul(
                        stt[:, idx * 512 + off:(idx + 1) * 512],
                        kt_s[hd:hd + 64, kg:kg + 128],
                        qt_s[hd:hd + 64, g0 + off:g0 + 512],
                        start=True, stop=not diag,
                    )
                    if diag:  # add -1e9 upper triangle on the PE
                        nc.tensor.matmul(
                            stt[:, idx * 512 + off:idx * 512 + off + 128],
                            ident, tri_r, start=False, stop=True,
                        )
                    offs.append(off)
                ptt = ptpool.tile([128, 1024], BF16, tag="pt",
                                  name=f"pt_{b}_{w}_{k0}_{h}")
                nc.scalar.activation(
                    ptt[:, offs[0]:1024], stt[:, offs[0]:1024],
                    AF.Exp, scale=SCALE,
                )
                sttd[k0, h] = (offs, ptt)

            def emit_av_pair(b, w, k0, h, avs, sttd, nk):
                offs, ptt = sttd.pop((k0, h))
                for idx in range(2):
                    ki = k0 + idx
                    off = offs[idx]
                    blk = (b * T) // 128 + ki
                    nc.tensor.matmul(
                        avs[h][0:65, off:512],
                        vtm[:, h, blk, :],
                        ptt[:, idx * 512 + off:(idx + 1) * 512],
                        start=(ki == 0), stop=(ki == nk - 1),
                    )

            def emit_tail(b, w, avs):
                """normalize by l, out-projection, store one 512-window."""
                g0 = b * T + w * WIN
                ot_w = otwp.tile([128, 512], BF16, tag="ot",
                                 name=f"ot_{b}_{w}")
                for h in range(H_LOC):
                    ap = avs[h]
                    lrow = m2.tile([1, 512], F32, tag="lrow",
                                   name=f"lr_{b}_{w}_{h}")
                    nc.vector.tensor_copy(lrow, ap[64:65, :])
                    lrinv = m2.tile([1, 512], F32, tag="lrinv",
                                    name=f"lv_{b}_{w}_{h}")
                    nc.vector.reciprocal_approx_fast(out=lrinv, in_=lrow)
                    linv = m2.tile([64, 512], F32, tag="linv",
                                   name=f"li_{b}_{w}_{h}")
                    nc.gpsimd.partition_broadcast(linv, lrinv)
                    nc.vector.tensor_mul(
                        ot_w[h * 64:h * 64 + 64, :], ap[0:64, :], linv
                    )
                for ti in range(4):
                    t0 = g0 + ti * 128
                    for co in range(2):
                        po = ps.tile([128, 512], F32, tag="qk", bufs=2,
                                     name=f"po_{b}_{w}_{ti}_{co}")
                        nc.tensor.matmul(
                            po, ot_w[:, ti * 128:ti * 128 + 128],
                            wp_sb[:, co * 512:(co + 1) * 512],
                            start=True, stop=True,
                        )
                        ob = obp.tile([128, 512], F32, tag="ob")
                        nc.vector.tensor_copy(ob, po)
                        nc.sync.dma_start(
                            out=outp[t0:t0 + 128, co * 512:(co + 1) * 512],
                            in_=ob,
                        )

            # ---------------- main interleaved schedule -------------------
            emit_x_dma(0)
            for c in range(4):
                qkv_chunk(0, c)

            windows = [(b, w) for b in range(B) for w in range(NW)]
            prev_tail = None
            for i, (b, w) in enumerate(windows):
                nk = 4 * (w + 1)
                npairs = nk // 2
                if i + 1 < TT:
                    emit_x_dma(i + 1)
                chunks = [(i + 1, c) for c in range(4)] if i + 1 < TT else []
                avs = {}
                sttd = {}
                pend = []
                for j in range(npairs):
                    k0 = 2 * j
                    for h in range(H_LOC):
                        emit_s_exp_pair(b, w, k0, h, sttd)
                    if chunks:
                        qkv_chunk(*chunks.pop(0))
                    if j == min(2, npairs - 1) and prev_tail is not None:
                        emit_tail(*prev_tail)
                        prev_tail = None
                    if j >= 1:
                        if not avs:
                            for h in range(H_LOC):
                                avs[h] = ps.tile(
                                    [128, 512], F32, tag="av", bufs=2,
                                    name=f"av_{b}_{w}_{h}",
                                )
                        pk0 = 2 * (j - 1)
                        for h in range(H_LOC):
                            emit_av_pair(b, w, pk0, h, avs, sttd, nk)
                if prev_tail is not None:   # npairs == 2 path never hits j>1
                    emit_tail(*prev_tail)
                    prev_tail = None
                if not avs:
                    for h in range(H_LOC):
                        avs[h] = ps.tile([128, 512], F32, tag="av", bufs=2,
                                         name=f"av_{b}_{w}_{h}")
                for h in range(H_LOC):
                    emit_av_pair(b, w, 2 * (npairs - 1), h, avs, sttd, nk)
                while chunks:
                    qkv_chunk(*chunks.pop(0))
                prev_tail = (b, w, avs)
            emit_tail(*prev_tail)
    nc.compile()
    return nc


_PROGRAM = None


def _get_program():
    global _PROGRAM
    if _PROGRAM is None:
        _PROGRAM = build_program()
    return _PROGRAM


def _make_in_maps(x, W_qkv, b_qkv, W_proj):
    B, T, C = x.shape
    xT = np.ascontiguousarray(
        x.reshape(B * T, C).T.astype(BF16_NP)
    )
    in_maps = []
    for c in range(N_CORES):
        lo, hi = c * D_LOC, (c + 1) * D_LOC
        in_maps.append({
            "xT": xT,
            "wq": np.ascontiguousarray(W_qkv[:, lo:hi].astype(BF16_NP)),
            "wk": np.ascontiguousarray(W_qkv[:, C + lo:C + hi].astype(BF16_NP)),
            "wv": np.ascontiguousarray(W_qkv[:, 2 * C + lo:2 * C + hi].astype(BF16_NP)),
            "bq": np.ascontiguousarray(b_qkv[lo:hi].reshape(-1, 1), np.float32),
            "bk": np.ascontiguousarray(b_qkv[C + lo:C + hi].reshape(-1, 1), np.float32),
            "bv": np.ascontiguousarray(b_qkv[2 * C + lo:2 * C + hi].reshape(-1, 1), np.float32),
            "wp": np.ascontiguousarray(W_proj[lo:hi, :].astype(BF16_NP)),
        })
    return in_maps


LAST_RESULT = None


def run(inputs, trace=False):
    """Returns (full output [B,T,C] float32, exec_time_ns or None)."""
    global LAST_RESULT
    x = np.asarray(inputs["x"], np.float32)
    W_qkv = np.asarray(inputs["W_qkv"], np.float32)
    b_qkv = np.asarray(inputs["b_qkv"], np.float32)
    W_proj = np.asarray(inputs["W_proj"], np.float32)
    b_proj = np.asarray(inputs["b_proj"], np.float32)
    B, T, C = x.shape

    nc = _get_program()
    in_maps = _make_in_maps(x, W_qkv, b_qkv, W_proj)
    res = run_bass_kernel_spmd(
        nc, in_maps, list(range(N_CORES)), trace=trace
    )
    LAST_RESULT = res
    acc = np.zeros((B * T, C), np.float64)
    for c in range(N_CORES):
        acc += res.results[c]["outp"].astype(np.float64)
    out = (acc + b_proj.astype(np.float64)).astype(np.float32)
    return out.reshape(B, T, C), res.exec_time_ns


def kernel(**inputs):
    out, _ = run(inputs, trace=False)
    return out


# revision 15
# speedup vs baseline: 1.6439x; 1.0343x over previous
"""Causal self-attention (B=4, T=2048, C=1024, H=16) on 8 trn2 NeuronCores.

Sharding: tensor-parallel over heads. Core c owns heads {2c, 2c+1}:
  - computes Q,K,V projections for its 2 heads (full batch/sequence),
  - causal attention for its heads,
  - a partial output projection (row-slice of W_proj),
and the host sums the 8 partial projections (+ b_proj).

v4 design (v1 baseline 731us, v3 551us):
  - q-windows of 512 with ki-PAIRED S^T PSUM tiles: stt [128,1024] holds two
    k-tiles' S (512 q columns each) so one ScalarE exp instruction covers a
    k-tile pair (halves ACT instruction overhead vs 512-wide exps) while a
    single stt costs 2 PSUM banks.  PSUM budget: st 2x2 + av 2 + qkv 2 = 8.
  - st bufs=2 gives 4 k-tiles of S/exp lookahead; AV matmuls (lhsT =
    token-major V_aug bf16 [tok,65]; ones column = softmax denominator) lag
    one pair behind so the PE never queues behind ACT.
  - QKV projection is interleaved INTO the attention loop: window i emits
    the QKV chunks for token-tile i+1 (matmul work that fills every PE
    dependency stall and keeps HAM at K=8/8 -- v1/v3 ran attention at
    1.2GHz because PE idle gaps kept it throttled).
  - S matmuls alternate heads (row groups 0-1/2-3 via base partitions) for
    PE row-tiling overlap; exp output is bf16; causal masking is a 0/1
    multiply on the diagonal 128-block after exp, alternating DVE/gpsimd;
    out-projection PSUM->SBUF copies alternate DVE/ScalarE.
  - normalize: l row -> base-0 tile (cross-base 1-partition DVE copy),
    reciprocal_approx_fast, gpsimd partition_broadcast, one DVE mul per
    head (DVE handles the head-1 base-64 output directly).
"""

import ml_dtypes
import numpy as np

import concourse.bacc as bacc

BF16_NP = ml_dtypes.bfloat16
import concourse.tile as tile
from concourse import mybir
from concourse.bass_utils import run_bass_kernel_spmd
from concourse.masks import make_identity

F32 = mybir.dt.float32
F32R = mybir.dt.float32r
BF16 = mybir.dt.bfloat16
AF = mybir.ActivationFunctionType
ALU = mybir.AluOpType

N_CORES = 8
D_MODEL = 1024
N_HEADS = 16
HEAD_DIM = 64
H_LOC = 2                  # heads per core
D_LOC = H_LOC * HEAD_DIM   # 128
SCALE = 1.0 / np.sqrt(HEAD_DIM)
WIN = 512                  # q-window width


def build_program(B=4, T=2048):
    TOK = B * T
    TT = TOK // 512          # tok tiles of 512 for the QKV matmul
    CT = D_MODEL // 128      # contraction tiles
    NW = T // WIN            # q-windows per batch
    assert T % WIN == 0 and TOK % 512 == 0

    nc = bacc.Bacc(
        "TRN2", target_bir_lowering=False, debug=False, num_devices=N_CORES
    )
    xT = nc.dram_tensor("xT", [D_MODEL, TOK], BF16, kind="ExternalInput").ap()
    wq = nc.dram_tensor("wq", [D_MODEL, D_LOC], BF16, kind="ExternalInput").ap()
    wk = nc.dram_tensor("wk", [D_MODEL, D_LOC], BF16, kind="ExternalInput").ap()
    wv = nc.dram_tensor("wv", [D_MODEL, D_LOC], BF16, kind="ExternalInput").ap()
    bq = nc.dram_tensor("bq", [D_LOC, 1], F32, kind="ExternalInput").ap()
    bk = nc.dram_tensor("bk", [D_LOC, 1], F32, kind="ExternalInput").ap()
    bv = nc.dram_tensor("bv", [D_LOC, 1], F32, kind="ExternalInput").ap()
    wp = nc.dram_tensor("wp", [D_LOC, D_MODEL], BF16, kind="ExternalInput").ap()
    outp = nc.dram_tensor("outp", [TOK, D_MODEL], BF16, kind="ExternalOutput").ap()

    with tile.TileContext(nc) as tc:
        with (
            tc.tile_pool(name="const", bufs=1) as const,
            tc.tile_pool(name="res", bufs=1) as res,
            tc.tile_pool(name="xst", bufs=10) as xst,
            tc.tile_pool(name="vtt", bufs=2) as vtt,
            tc.tile_pool(name="ptp", bufs=6) as ptpool,
            tc.tile_pool(name="m2", bufs=2) as m2,
            tc.tile_pool(name="otw", bufs=2) as otwp,
            tc.tile_pool(name="ob", bufs=4) as obp,
            tc.tile_pool(name="ps", bufs=1, space="PSUM") as ps,
        ):
            # --- constants -------------------------------------------------
            wq_sb = const.tile([128, CT, D_LOC], BF16, tag="wq")
            wk_sb = const.tile([128, CT, D_LOC], BF16, tag="wk")
            wv_sb = const.tile([128, CT, D_LOC], BF16, tag="wv")
            for w_sb, w_dram in ((wq_sb, wq), (wk_sb, wk), (wv_sb, wv)):
                nc.sync.dma_start(
                    out=w_sb, in_=w_dram.rearrange("(ct p) d -> p ct d", p=128)
                )
            wp_sb = const.tile([128, D_MODEL], BF16, tag="wp")
            nc.sync.dma_start(out=wp_sb, in_=wp)
            bq_sb = const.tile([128, 1], F32, tag="bq")
            bk_sb = const.tile([128, 1], F32, tag="bk")
            bv_sb = const.tile([128, 1], F32, tag="bv")
            for b_sb, b_dram in ((bq_sb, bq), (bk_sb, bk), (bv_sb, bv)):
                nc.sync.dma_start(out=b_sb, in_=b_dram)

            # causal mask for a diagonal S^T block [k, q]: 0 where q >= k,
            # -1e9 where q < k.  Applied ON THE PE as an accumulated matmul
            # (ident.T @ tri) so the hot loop never round-trips DVE/gpsimd.
            mask_f32 = const.tile([128, 128], F32, tag="mask_f32")
            nc.gpsimd.memset(mask_f32, 0.0)
            nc.gpsimd.affine_select(
                out=mask_f32, in_=mask_f32, compare_op=ALU.is_ge,
                fill=-1.0e9, base=0, pattern=[[1, 128]], channel_multiplier=-1,
            )
            tri_r = const.tile([128, 128], BF16, tag="tri_r")
            nc.vector.tensor_copy(tri_r, mask_f32)
            ident_f32 = const.tile([128, 128], F32, tag="ident_f32")
            make_identity(nc, ident_f32)
            ident = const.tile([128, 128], BF16, tag="ident")
            nc.vector.tensor_copy(ident, ident_f32)
            ones_f32 = const.tile([128, 128], F32, tag="ones_f32")
            nc.vector.memset(ones_f32, 1.0)

            # --- resident tensors -----------------------------------------
            qt_s = res.tile([128, TOK], BF16, tag="qt")   # [d(2 heads), tok]
            kt_s = res.tile([128, TOK], BF16, tag="kt")
            # token-major V with ones column: [tok(128), head, blk, 65] bf16
            vtm = res.tile([128, H_LOC, TOK // 128, 65], BF16, tag="vtm")
            nc.vector.tensor_copy(
                vtm[:, :, :, 64],
                ones_f32.rearrange("p (h b) -> p h b", h=H_LOC)[:, :, :TOK // 128],
            )

            # ---------------- QKV chunk machinery -------------------------
            # Each token-tile tt = 512 tokens; 4 chunks: q / k / v+vt / 4
            # transposes+vtm copy.  PSUM tag "qk" (2 banks) is exclusive to
            # this machinery, so it never deadlocks against st/av rings.
            xs_tiles = {}

            def emit_x_dma(tt):
                t0 = tt * 512
                xs = []
                for ct in range(CT):
                    xt = xst.tile([128, 512], BF16, tag="x",
                                  name=f"x_{tt}_{ct}")
                    nc.sync.dma_start(
                        out=xt, in_=xT[ct * 128:(ct + 1) * 128, t0:t0 + 512]
                    )
                    xs.append(xt)
                xs_tiles[tt] = xs

            xs_vt = {}

            def qkv_chunk(tt, c):
                t0 = tt * 512
                xs = xs_tiles[tt]
                if c < 3:
                    w_sb = (wq_sb, wk_sb, wv_sb)[c]
                    acc = ps.tile([128, 512], F32, tag="qk", bufs=2,
                                  name=f"p{'qkv'[c]}_{tt}")
                    for ct in range(CT):
                        nc.tensor.matmul(acc, w_sb[:, ct, :], xs[ct],
                                         start=ct == 0, stop=ct == CT - 1)
                    if c == 0:
                        nc.vector.tensor_scalar_add(
                            qt_s[:, t0:t0 + 512], acc, bq_sb)
                    elif c == 1:
                        nc.vector.tensor_scalar_add(
                            kt_s[:, t0:t0 + 512], acc, bk_sb)
                    else:
                        vt = vtt.tile([128, 512], BF16, tag="vt",
                                      name=f"vt_{tt}")
                        nc.vector.tensor_scalar_add(vt, acc, bv_sb)
                        xs_vt[tt] = vt
                else:
                    vt = xs_vt.pop(tt)
                    tp4 = ps.tile([128, 512], BF16, tag="qk", bufs=2,
                                  name=f"tp_{tt}")
                    for j in range(4):
                        nc.tensor.transpose(
                            tp4[:, j * 128:(j + 1) * 128],
                            vt[:, j * 128:(j + 1) * 128], ident,
                        )
                    blk0 = tt * 4
                    nc.vector.tensor_copy(
                        vtm[:, :, blk0:blk0 + 4, 0:64],
                        tp4.rearrange("p (j h d) -> p h j d", j=4, h=H_LOC),
                    )
                    del xs_tiles[tt]

            # ---------------- attention window machinery ------------------
            def emit_s_exp_pair(b, w, k0, sttd):
                """S matmuls for k-tiles (k0, k0+1), BOTH heads, head-
                alternated so consecutive MMs land in different PE row
                groups and overlap; then the diagonal -1e9 triangles (full-
                row MMs, deferred), then one exp per head."""
                g0 = b * T + w * WIN
                stts, offs, tris = {}, [], []
                for h in range(H_LOC):
                    stts[h] = ps.tile([128, 1024], F32, tag="st", bufs=2,
                                      name=f"st_{b}_{w}_{k0}_{h}")
                for idx in range(2):
                    ki = k0 + idx
                    off = max(0, ki * 128 - w * WIN)
                    kg = b * T + ki * 128
                    diag = ki >= 4 * w
                    for h in range(H_LOC):
                        hd = h * 64
                        nc.tensor.matmul(
                            stts[h][:, idx * 512 + off:(idx + 1) * 512],
                            kt_s[hd:hd + 64, kg:kg + 128],
                            qt_s[hd:hd + 64, g0 + off:g0 + 512],
                            start=True, stop=not diag,
                        )
                        if diag:
                            tris.append((h, idx * 512 + off))
                    offs.append(off)
                for (h, lo) in tris:
                    nc.tensor.matmul(
                        stts[h][:, lo:lo + 128],
                        ident, tri_r, start=False, stop=True,
                    )
                for h in range(H_LOC):
                    ptt = ptpool.tile([128, 1024], BF16, tag="pt",
                                      name=f"pt_{b}_{w}_{k0}_{h}")
                    nc.scalar.activation(
                        ptt[:, offs[0]:1024], stts[h][:, offs[0]:1024],
                        AF.Exp, scale=SCALE,
                    )
                    sttd[k0, h] = (offs, ptt)

            def emit_av_pair(b, w, k0, h, avs, sttd, nk):
                offs, ptt = sttd.pop((k0, h))
                for idx in range(2):
                    ki = k0 + idx
                    off = offs[idx]
                    blk = (b * T) // 128 + ki
                    nc.tensor.matmul(
                        avs[h][0:65, off:512],
                        vtm[:, h, blk, :],
                        ptt[:, idx * 512 + off:(idx + 1) * 512],
                        start=(ki == 0), stop=(ki == nk - 1),
                    )

            def emit_tail(b, w, avs):
                """normalize by l, out-projection, store one 512-window."""
                g0 = b * T + w * WIN
                ot_w = otwp.tile([128, 512], BF16, tag="ot",
                                 name=f"ot_{b}_{w}")
                for h in range(H_LOC):
                    ap = avs[h]
                    lrow = m2.tile([1, 512], F32, tag="lrow",
                                   name=f"lr_{b}_{w}_{h}")
                    nc.vector.tensor_copy(lrow, ap[64:65, :])
                    lrinv = m2.tile([1, 512], F32, tag="lrinv",
                                    name=f"lv_{b}_{w}_{h}")
                    nc.vector.reciprocal_approx_fast(out=lrinv, in_=lrow)
                    linv = m2.tile([64, 512], F32, tag="linv",
                                   name=f"li_{b}_{w}_{h}")
                    nc.gpsimd.partition_broadcast(linv, lrinv)
                    nc.vector.tensor_mul(
                        ot_w[h * 64:h * 64 + 64, :], ap[0:64, :], linv
                    )
                for ti in range(4):
                    t0 = g0 + ti * 128
                    for co in range(2):
                        po = ps.tile([128, 512], F32, tag="qk", bufs=2,
                                     name=f"po_{b}_{w}_{ti}_{co}")
                        nc.tensor.matmul(
                            po, ot_w[:, ti * 128:ti * 128 + 128],
                            wp_sb[:, co * 512:(co + 1) * 512],
                            start=True, stop=True,
                        )
                        ob = obp.tile([128, 512], BF16, tag="ob")
                        nc.vector.tensor_copy(ob, po)
                        nc.sync.dma_start(
                            out=outp[t0:t0 + 128, co * 512:(co + 1) * 512],
                            in_=ob,
                        )

            # ---------------- main interleaved schedule -------------------
            emit_x_dma(0)
            for c in range(4):
                qkv_chunk(0, c)

            windows = [(b, w) for b in range(B) for w in range(NW)]
            prev_tail = None
            for i, (b, w) in enumerate(windows):
                nk = 4 * (w + 1)
                npairs = nk // 2
                if i + 1 < TT:
                    emit_x_dma(i + 1)
                chunks = [(i + 1, c) for c in range(4)] if i + 1 < TT else []
                avs = {}
                sttd = {}
                pend = []
                for j in range(npairs):
                    k0 = 2 * j
                    emit_s_exp_pair(b, w, k0, sttd)
                    if chunks:
                        qkv_chunk(*chunks.pop(0))
                    if j == min(2, npairs - 1) and prev_tail is not None:
                        emit_tail(*prev_tail)
                        prev_tail = None
                    if j >= 1:
                        if not avs:
                            for h in range(H_LOC):
                                avs[h] = ps.tile(
                                    [128, 512], F32, tag="av", bufs=2,
                                    name=f"av_{b}_{w}_{h}",
                                )
                        pk0 = 2 * (j - 1)
                        for h in range(H_LOC):
                            emit_av_pair(b, w, pk0, h, avs, sttd, nk)
                if prev_tail is not None:   # npairs == 2 path never hits j>1
                    emit_tail(*prev_tail)
                    prev_tail = None
                if not avs:
                    for h in range(H_LOC):
                        avs[h] = ps.tile([128, 512], F32, tag="av", bufs=2,
                                         name=f"av_{b}_{w}_{h}")
                for h in range(H_LOC):
                    emit_av_pair(b, w, 2 * (npairs - 1), h, avs, sttd, nk)
                while chunks:
                    qkv_chunk(*chunks.pop(0))
                prev_tail = (b, w, avs)
            emit_tail(*prev_tail)
    nc.compile()
    return nc


_PROGRAM = None


def _get_program():
    global _PROGRAM
    if _PROGRAM is None:
        _PROGRAM = build_program()
    return _PROGRAM


def _make_in_maps(x, W_qkv, b_qkv, W_proj):
    B, T, C = x.shape
    xT = np.ascontiguousarray(
        x.reshape(B * T, C).T.astype(BF16_NP)
    )
    in_maps = []
    for c in range(N_CORES):
        lo, hi = c * D_LOC, (c + 1) * D_LOC
        in_maps.append({
            "xT": xT,
            "wq": np.ascontiguousarray(W_qkv[:, lo:hi].astype(BF16_NP)),
            "wk": np.ascontiguousarray(W_qkv[:, C + lo:C + hi].astype(BF16_NP)),
            "wv": np.ascontiguousarray(W_qkv[:, 2 * C + lo:2 * C + hi].astype(BF16_NP)),
            "bq": np.ascontiguousarray(b_qkv[lo:hi].reshape(-1, 1), np.float32),
            "bk": np.ascontiguousarray(b_qkv[C + lo:C + hi].reshape(-1, 1), np.float32),
            "bv": np.ascontiguousarray(b_qkv[2 * C + lo:2 * C + hi].reshape(-1, 1), np.float32),
            "wp": np.ascontiguousarray(W_proj[lo:hi, :].astype(BF16_NP)),
        })
    return in_maps


LAST_RESULT = None


def run(inputs, trace=False):
    """Returns (full output [B,T,C] float32, exec_time_ns or None)."""
    global LAST_RESULT
    x = np.asarray(inputs["x"], np.float32)
    W_qkv = np.asarray(inputs["W_qkv"], np.float32)
    b_qkv = np.asarray(inputs["b_qkv"], np.float32)
    W_proj = np.asarray(inputs["W_proj"], np.float32)
    b_proj = np.asarray(inputs["b_proj"], np.float32)
    B, T, C = x.shape

    nc = _get_program()
    in_maps = _make_in_maps(x, W_qkv, b_qkv, W_proj)
    res = run_bass_kernel_spmd(
        nc, in_maps, list(range(N_CORES)), trace=trace
    )
    LAST_RESULT = res
    acc = np.zeros((B * T, C), np.float64)
    for c in range(N_CORES):
        acc += res.results[c]["outp"].astype(np.float64)
    out = (acc + b_proj.astype(np.float64)).astype(np.float32)
    return out.reshape(B, T, C), res.exec_time_ns


def kernel(**inputs):
    out, _ = run(inputs, trace=False)
    return out
